# revision 19
# baseline (speedup 1.0000x reference)
"""AttnTopKPool Trainium2 kernel, v5: 4-slot DVE/GpSimd-added column sums.

reference:
    w_mean = mean(w, axis=1)          # [B, S, S] -> [B, S]
    idx    = top_k(w_mean, 16)        # [B, 16]
    out    = x[b, :, idx[b]]          # [B, F, 16]

Strategy (8 NeuronCores, batch-parallel, 4 batches each):
  - host: transpose x to x_t[b, s, f]; slice w and x_t per core.
  - device per batch (16 MiB of w streamed once at ~425 GB/s):
      * 16 uniform 1 MiB quarter loads [128, 2048] (4 slots x 4 fr,
        rows 512t+4p+fr) on the sync HWDGE queue.
      * acc = ((w0+w1)+w2)+w3 via chunk adds: add1 on GpSimd (off the
        matmul-gating chain), add2/add3 on DVE right behind the w2/w3
        quarter landings. This summation order was validated on hardware
        to reproduce the jax fp32 reference top-16 ranking on all 32
        batches (near-tied column sums make the ranking sensitive to
        the exact fp32 rounding; see sums_experiment.py).
      * column sums via 16 fp32 ones-matmuls (PSUM accumulated, 4 banks);
        fp32r would be 4x faster but mis-ranks two batches - verified.
      * top-16 via DVE max8 / max_index / match_replace, two rounds,
        software-pipelined one batch behind so the vector queue never
        head-of-line-blocks the adds; gathers issue per round.
      * gather: per index, reg_load into a register and issue a
        dynamic-offset HWDGE DMA copying that 4 KiB row of x_t[b]
        straight DRAM->DRAM into the output row.
  - out per core: [64, 1024] = (b_loc*16 + k, f); host reassembles.
"""

import numpy as np

B, F, S, K = 32, 1024, 2048, 16
N_CORES = 8
B_LOC = B // N_CORES  # 4
P = 128
MM_N = 512                 # fp32 moving-operand max / one PSUM bank
NQ = S // MM_N             # 4 psum column slices
FR = 4                     # w rows per partition in a slot
NEG = -3.0e38              # below any column sum

_cached_nc = None

# test-only knobs (harness leaves these at defaults)
TRACE = False
_last_results = None


def _build_nc():
    from concourse import bacc, bass, mybir, tile

    f32 = mybir.dt.float32
    u32 = mybir.dt.uint32

    nc = bacc.Bacc("TRN2", target_bir_lowering=False, debug=False)

    w_d = nc.dram_tensor("w", [B_LOC, S, S], f32, kind="ExternalInput")
    xt_d = nc.dram_tensor("xt", [B_LOC, S, F], f32, kind="ExternalInput")
    out_d = nc.dram_tensor("out", [B_LOC * K, F], f32, kind="ExternalOutput")

    w_rows = w_d[:].rearrange("b r s -> (b r) s")
    # quarter view: [16, 4, 128, 2048]; [x, fr] partition p holds row 512x+4p+fr
    w_q = w_rows.rearrange("(x p fr) s -> x fr p s", p=P, fr=FR)

    with tile.TileContext(nc) as tc:
        with (
            tc.tile_pool(name="qpool", bufs=5) as qpool,
            tc.tile_pool(name="smpool", bufs=2) as smpool,
            tc.tile_pool(name="pspool", bufs=2, space="PSUM") as pspool,
            tc.tile_pool(name="tk", bufs=1) as tk,
        ):
            ones = tk.tile([P, 1], f32)
            nc.vector.memset(ones[:], 1.0)

            def topk_and_gather(b, sums):
                """Two-round top-16 on DVE; gathers issue per round via
                dynamic-offset HWDGE DMAs straight DRAM->DRAM."""
                gidx_a = tk.tile([1, 8], u32, name=f"gidxa{b}", tag="gidxa", bufs=2)
                gidx_b = tk.tile([1, 8], u32, name=f"gidxb{b}", tag="gidxb", bufs=2)
                m8a = tk.tile([1, 8], f32, name=f"m8a{b}", tag="m8a", bufs=2)
                m8b = tk.tile([1, 8], f32, name=f"m8b{b}", tag="m8b", bufs=2)
                nc.vector.max(m8a[:], sums[:])
                nc.vector.max_index(gidx_a[:], m8a[:], sums[:])

                def gather(k, gidx, eng, etype):
                    regs = nc.alloc_registers(name=f"ri{b}_{k}", engines=(etype,))
                    reg = list(regs)[0]
                    eng.reg_load(reg, gidx[0:1, k % 8 : k % 8 + 1])
                    val = eng.snap(reg, donate=True, min_val=0, max_val=S - 1)
                    eng.dma_start(
                        out_d[b * K + k : b * K + k + 1, :],
                        xt_d[b][bass.ds(val, 1), :],
                    )

                # on scalar (HWDGE) so the sync w-load queue never stalls
                # behind a top-k-dependent reg_load; the final batch has no
                # w stream left, so its gathers also split onto sync.
                def dispatch(k, gidx):
                    if b == B_LOC - 1 and k % 2 == 1:
                        gather(k, gidx, nc.sync, mybir.EngineType.SP)
                    else:
                        gather(k, gidx, nc.scalar, mybir.EngineType.Activation)

                for k in range(8):
                    dispatch(k, gidx_a)

                nc.vector.match_replace(sums[:], m8a[:], sums[:], NEG)
                nc.vector.max(m8b[:], sums[:])
                nc.vector.max_index(gidx_b[:], m8b[:], sums[:])
                for k in range(8, 16):
                    dispatch(k, gidx_b)

            prev = None  # (b, sums) whose top-k is deferred one batch
            for b in range(B_LOC):
                # --- stream w[b]: 16 x 1 MiB quarter loads, sync queue ---
                w0 = [
                    qpool.tile([P, S], f32, name=f"w0_{b}_{fr}", tag="w0", bufs=6)
                    for fr in range(FR)
                ]
                w1 = [
                    qpool.tile([P, S], f32, name=f"w1_{b}_{fr}", tag="w1")
                    for fr in range(FR)
                ]
                w2 = [
                    qpool.tile([P, S], f32, name=f"w2_{b}_{fr}", tag="w2")
                    for fr in range(FR)
                ]
                w3 = [
                    qpool.tile([P, S], f32, name=f"w3_{b}_{fr}", tag="w3")
                    for fr in range(FR)
                ]
                for fr in range(FR):
                    nc.sync.dma_start(w0[fr][:], w_q[4 * b + 0, fr])
                for fr in range(FR):
                    nc.sync.dma_start(w1[fr][:], w_q[4 * b + 1, fr])
                for fr in range(FR):
                    nc.sync.dma_start(w2[fr][:], w_q[4 * b + 2, fr])
                for fr in range(FR):
                    nc.sync.dma_start(w3[fr][:], w_q[4 * b + 3, fr])

                # previous batch's top-k first: its inputs are ready early
                # in this batch's stream window while DVE is otherwise idle
                # (add1 is on GpSimd; add2 only starts once w2 lands).
                if prev is not None:
                    topk_and_gather(*prev)

                # --- chunk adds: acc = ((w0+w1)+w2)+w3, elementwise ---
                for fr in range(FR):
                    nc.gpsimd.tensor_add(w0[fr][:], w0[fr][:], w1[fr][:])
                for fr in range(FR):
                    nc.vector.tensor_add(w0[fr][:], w0[fr][:], w2[fr][:])
                for fr in range(FR):
                    nc.vector.tensor_add(w0[fr][:], w0[fr][:], w3[fr][:])

                ps = [
                    pspool.tile([1, MM_N], f32, name=f"ps{b}_{q}", tag=f"ps{q}")
                    for q in range(NQ)
                ]
                # single accumulation group per psum slice; WAW deps on the
                # psum AP keep the start=True matmul first
                for c in range(FR * NQ):
                    fr, q = c // NQ, c % NQ
                    nc.tensor.matmul(
                        ps[q][:],
                        ones[:],
                        w0[fr][:, q * MM_N : (q + 1) * MM_N],
                        start=(c < NQ),
                        stop=(c >= FR * NQ - NQ),
                    )

                # PSUM -> column sums in SBUF
                sums = smpool.tile([1, S], f32, name=f"sums{b}", tag="sums")
                for q in range(NQ):
                    nc.scalar.activation(
                        sums[:, q * MM_N : (q + 1) * MM_N],
                        ps[q][:],
                        mybir.ActivationFunctionType.Copy,
                    )
                prev = (b, sums)

            # last batch's top-k + gathers are the kernel tail
            topk_and_gather(*prev)

    nc.compile()
    return nc


def _get_nc():
    global _cached_nc
    if _cached_nc is None:
        _cached_nc = _build_nc()
    return _cached_nc


def kernel(x: np.ndarray, w: np.ndarray) -> np.ndarray:
    from concourse import bass_utils

    x = np.asarray(x, dtype=np.float32)
    w = np.asarray(w, dtype=np.float32)
    x_t = np.ascontiguousarray(x.transpose(0, 2, 1))  # [B, S, F]

    nc = _get_nc()
    in_maps = [
        {
            "w": np.ascontiguousarray(w[c * B_LOC : (c + 1) * B_LOC]),
            "xt": x_t[c * B_LOC : (c + 1) * B_LOC],
        }
        for c in range(N_CORES)
    ]
    res = bass_utils.run_bass_kernel_spmd(
        nc, in_maps, list(range(N_CORES)), trace=TRACE
    )
    global _last_results
    _last_results = res
    out = np.concatenate([res.results[c]["out"] for c in range(N_CORES)], axis=0)
    # [B*K, F] -> [B, K, F] -> [B, F, K]
    return np.ascontiguousarray(out.reshape(B, K, F).transpose(0, 2, 1))


# revision 20
# speedup vs baseline: 1.0289x; 1.0289x over previous
"""AttnTopKPool Trainium2 kernel.

reference:
    w_mean = mean(w, axis=1)          # [B, S, S] -> [B, S]
    idx    = top_k(w_mean, 16)        # [B, 16]
    out    = x[b, :, idx[b]]          # [B, F, 16]

Strategy (8 NeuronCores, batch-parallel, 4 batches each):
  - host: transpose x to x_t[b, s, f] so the device gather is a contiguous
    row gather; slice w and x_t per core.
  - device per batch (w[b] is 16 MiB, streamed once; memory-bound):
      * 16 uniform 1 MiB loads [128, 2048]: three "big slots" worth of
        quarter tiles (wt0q/wt1q/wt2q, rows 512t+4p+fr) plus four
        partition-major small tiles (rows 1536+128m+p).
      * DVE chunk-adds pipelined with the stream: wt0q[fr] += wt1q[fr]
        as wt1 quarters land, then += wt2q[fr] as wt2 quarters land.
        Element-wise order is bit-identical to the previous whole-tile
        adds (this matters: several batches have near-tied column sums
        whose ordering under fp32 rounding must reproduce the
        reference's top_k exactly).
      * column sums via TensorE fp32 ones-matmul into 4 PSUM banks,
        accumulation order identical to the reference-passing schedule:
        16 slices over the pre-added quarters, then 16 over smalls.
      * top-16 via DVE max8 / max_index / match_replace (two rounds);
        gathers for ranks 0-7 issue as soon as round 1 lands.
      * gather: per index, reg_load into an SP register and issue a
        dynamic-offset HWDGE DMA copying that 4 KiB row of x_t[b]
        straight DRAM->DRAM into the output row.
  - out per core: [64, 1024] = (b_loc*16 + k, f); host reassembles to
    [B, F, K].
"""

import numpy as np

B, F, S, K = 32, 1024, 2048, 16
N_CORES = 8
B_LOC = B // N_CORES  # 4
P = 128
MM_N = 512                 # fp32 moving-operand max / one PSUM bank
NQ = S // MM_N             # 4 psum column slices
FR = 4                     # w rows per partition in a big slot
NEG = -3.0e38              # below any column sum

_cached_nc = None

# test-only knobs (harness leaves these at defaults)
TRACE = False
_last_results = None


def _build_nc():
    from concourse import bacc, bass, mybir, tile

    f32 = mybir.dt.float32
    u32 = mybir.dt.uint32

    nc = bacc.Bacc("TRN2", target_bir_lowering=False, debug=False)

    w_d = nc.dram_tensor("w", [B_LOC, S, S], f32, kind="ExternalInput")
    xt_d = nc.dram_tensor("xt", [B_LOC, S, F], f32, kind="ExternalInput")
    out_d = nc.dram_tensor("out", [B_LOC * K, F], f32, kind="ExternalOutput")

    w_rows = w_d[:].rearrange("b r s -> (b r) s")
    # quarter view: [32, 4, 128, 2048]; [x, fr] partition p holds row 512x+4p+fr
    w_q = w_rows.rearrange("(x p fr) s -> x fr p s", p=P, fr=FR)
    # small view: [64, 128, 2048]; partition p of slot m holds row (128m + p)
    w_small = w_rows.rearrange("(m p) s -> m p s", p=P)

    with tile.TileContext(nc) as tc:
        with (
            tc.tile_pool(name="qpool", bufs=5) as qpool,
            tc.tile_pool(name="smpool", bufs=2) as smpool,
            tc.tile_pool(name="pspool", bufs=2, space="PSUM") as pspool,
            tc.tile_pool(name="tk", bufs=1) as tk,
        ):
            ones = tk.tile([P, 1], f32)
            nc.vector.memset(ones[:], 1.0)

            def topk_and_gather(b, sums):
                """Two-round top-16 on DVE; gathers issue per round via
                dynamic-offset HWDGE DMAs straight DRAM->DRAM."""
                gidx_a = tk.tile([1, 8], u32, name=f"gidxa{b}", tag="gidxa", bufs=2)
                gidx_b = tk.tile([1, 8], u32, name=f"gidxb{b}", tag="gidxb", bufs=2)
                m8a = tk.tile([1, 8], f32, name=f"m8a{b}", tag="m8a", bufs=2)
                m8b = tk.tile([1, 8], f32, name=f"m8b{b}", tag="m8b", bufs=2)
                nc.vector.max(m8a[:], sums[:])
                nc.vector.max_index(gidx_a[:], m8a[:], sums[:])

                def gather(k, gidx, eng, etype):
                    regs = nc.alloc_registers(name=f"ri{b}_{k}", engines=(etype,))
                    reg = list(regs)[0]
                    eng.reg_load(reg, gidx[0:1, k % 8 : k % 8 + 1])
                    val = eng.snap(reg, donate=True, min_val=0, max_val=S - 1)
                    eng.dma_start(
                        out_d[b * K + k : b * K + k + 1, :],
                        xt_d[b][bass.ds(val, 1), :],
                    )

                # on scalar (HWDGE) so the sync w-load queue never stalls
                # behind a top-k-dependent reg_load; the final batch has no
                # w stream or adds left, so its gathers also split onto the
                # idle sync and gpsimd queues.
                def dispatch(k, gidx):
                    if b == B_LOC - 1 and k % 2 == 1:
                        gather(k, gidx, nc.sync, mybir.EngineType.SP)
                    else:
                        gather(k, gidx, nc.scalar, mybir.EngineType.Activation)

                for k in range(8):
                    dispatch(k, gidx_a)

                nc.vector.match_replace(sums[:], m8a[:], sums[:], NEG)
                nc.vector.max(m8b[:], sums[:])
                nc.vector.max_index(gidx_b[:], m8b[:], sums[:])
                for k in range(8, 16):
                    dispatch(k, gidx_b)

            prev = None  # (b, sums) whose top-k is deferred one batch
            for b in range(B_LOC):
                # --- stream w[b]: 16 x 1 MiB loads on the sync queue ---
                w0 = [
                    qpool.tile([P, S], f32, name=f"w0_{b}_{fr}", tag="w0")
                    for fr in range(FR)
                ]
                w1 = [
                    qpool.tile([P, S], f32, name=f"w1_{b}_{fr}", tag="w1")
                    for fr in range(FR)
                ]
                w2 = [
                    qpool.tile([P, S], f32, name=f"w2_{b}_{fr}", tag="w2")
                    for fr in range(FR)
                ]
                sts = [
                    qpool.tile([P, S], f32, name=f"st{b}_{m}", tag="st")
                    for m in range(4)
                ]
                for fr in range(FR):
                    nc.sync.dma_start(w0[fr][:], w_q[4 * b + 0, fr])
                for fr in range(FR):
                    nc.sync.dma_start(w1[fr][:], w_q[4 * b + 1, fr])
                for fr in range(FR):
                    nc.sync.dma_start(w2[fr][:], w_q[4 * b + 2, fr])
                for m in range(4):
                    nc.sync.dma_start(sts[m][:], w_small[b * 16 + 12 + m])

                # --- DVE chunk-adds, pipelined behind the quarter DMAs ---
                # (element-wise identical to acc = w0+w1; acc += w2; gpsimd
                # adds were tried and are 2x slower per op, which delays the
                # matmul-gating chain)
                for fr in range(FR):
                    nc.vector.tensor_add(w0[fr][:], w0[fr][:], w1[fr][:])
                for fr in range(FR):
                    nc.vector.tensor_add(w0[fr][:], w0[fr][:], w2[fr][:])

                # previous batch's top-k goes on the vector queue only now,
                # behind this batch's adds: its wait for the previous MM
                # chain must not head-of-line-block the adds that gate this
                # batch's matmuls (and hence the buffer frees the DMA
                # stream needs).
                if prev is not None:
                    topk_and_gather(*prev)

                ps = [
                    pspool.tile([1, MM_N], f32, name=f"ps{b}_{q}", tag=f"ps{q}")
                    for q in range(NQ)
                ]
                # single accumulation group per psum slice; WAW deps on the
                # psum AP keep the start=True matmul first
                for c in range(FR * NQ):
                    fr, q = c // NQ, c % NQ
                    nc.tensor.matmul(
                        ps[q][:],
                        ones[:],
                        w0[fr][:, q * MM_N : (q + 1) * MM_N],
                        start=(c < NQ),
                        stop=False,
                    )
                for m, st in enumerate(sts):
                    for q in range(NQ):
                        nc.tensor.matmul(
                            ps[q][:],
                            ones[:],
                            st[:, q * MM_N : (q + 1) * MM_N],
                            start=False,
                            stop=(m == 3),
                        )
    
                # PSUM -> column sums in SBUF
                sums = smpool.tile([1, S], f32, name=f"sums{b}", tag="sums")
                for q in range(NQ):
                    nc.scalar.activation(
                        sums[:, q * MM_N : (q + 1) * MM_N],
                        ps[q][:],
                        mybir.ActivationFunctionType.Copy,
                    )

                prev = (b, sums)

            # last batch's top-k + gathers are the kernel tail
            topk_and_gather(*prev)

    nc.compile()
    return nc


def _get_nc():
    global _cached_nc
    if _cached_nc is None:
        _cached_nc = _build_nc()
    return _cached_nc


def kernel(x: np.ndarray, w: np.ndarray) -> np.ndarray:
    from concourse import bass_utils

    x = np.asarray(x, dtype=np.float32)
    w = np.asarray(w, dtype=np.float32)
    x_t = np.ascontiguousarray(x.transpose(0, 2, 1))  # [B, S, F]

    nc = _get_nc()
    in_maps = [
        {
            "w": np.ascontiguousarray(w[c * B_LOC : (c + 1) * B_LOC]),
            "xt": x_t[c * B_LOC : (c + 1) * B_LOC],
        }
        for c in range(N_CORES)
    ]
    res = bass_utils.run_bass_kernel_spmd(
        nc, in_maps, list(range(N_CORES)), trace=TRACE
    )
    global _last_results
    _last_results = res
    out = np.concatenate([res.results[c]["out"] for c in range(N_CORES)], axis=0)
    # [B*K, F] -> [B, K, F] -> [B, F, K]
    return np.ascontiguousarray(out.reshape(B, K, F).transpose(0, 2, 1))


# revision 21
# speedup vs baseline: 1.0304x; 1.0014x over previous
"""AttnTopKPool Trainium2 kernel.

reference:
    w_mean = mean(w, axis=1)          # [B, S, S] -> [B, S]
    idx    = top_k(w_mean, 16)        # [B, 16]
    out    = x[b, :, idx[b]]          # [B, F, 16]

Strategy (8 NeuronCores, batch-parallel, 4 batches each):
  - host: transpose x to x_t[b, s, f] so the device gather is a contiguous
    row gather; slice w and x_t per core.
  - device per batch (w[b] is 16 MiB, streamed once; memory-bound):
      * 16 uniform 1 MiB loads [128, 2048]: three "big slots" worth of
        quarter tiles (wt0q/wt1q/wt2q, rows 512t+4p+fr) plus four
        partition-major small tiles (rows 1536+128m+p).
      * DVE chunk-adds pipelined with the stream: wt0q[fr] += wt1q[fr]
        as wt1 quarters land, then += wt2q[fr] as wt2 quarters land.
        Element-wise order is bit-identical to the previous whole-tile
        adds (this matters: several batches have near-tied column sums
        whose ordering under fp32 rounding must reproduce the
        reference's top_k exactly).
      * column sums via TensorE fp32 ones-matmul into 4 PSUM banks,
        accumulation order identical to the reference-passing schedule:
        16 slices over the pre-added quarters, then 16 over smalls.
      * top-16 via DVE max8 / max_index / match_replace (two rounds);
        gathers for ranks 0-7 issue as soon as round 1 lands.
      * gather: per index, reg_load into an SP register and issue a
        dynamic-offset HWDGE DMA copying that 4 KiB row of x_t[b]
        straight DRAM->DRAM into the output row.
  - out per core: [64, 1024] = (b_loc*16 + k, f); host reassembles to
    [B, F, K].
"""

import numpy as np

B, F, S, K = 32, 1024, 2048, 16
N_CORES = 8
B_LOC = B // N_CORES  # 4
P = 128
MM_N = 512                 # fp32 moving-operand max / one PSUM bank
NQ = S // MM_N             # 4 psum column slices
FR = 4                     # w rows per partition in a big slot
NEG = -3.0e38              # below any column sum

_cached_nc = None

# test-only knobs (harness leaves these at defaults)
TRACE = False
_last_results = None


def _build_nc():
    from concourse import bacc, bass, mybir, tile

    f32 = mybir.dt.float32
    u32 = mybir.dt.uint32

    nc = bacc.Bacc("TRN2", target_bir_lowering=False, debug=False)

    w_d = nc.dram_tensor("w", [B_LOC, S, S], f32, kind="ExternalInput")
    xt_d = nc.dram_tensor("xt", [B_LOC, S, F], f32, kind="ExternalInput")
    out_d = nc.dram_tensor("out", [B_LOC * K, F], f32, kind="ExternalOutput")

    w_rows = w_d[:].rearrange("b r s -> (b r) s")
    # quarter view: [32, 4, 128, 2048]; [x, fr] partition p holds row 512x+4p+fr
    w_q = w_rows.rearrange("(x p fr) s -> x fr p s", p=P, fr=FR)
    # small view: [64, 128, 2048]; partition p of slot m holds row (128m + p)
    w_small = w_rows.rearrange("(m p) s -> m p s", p=P)

    with tile.TileContext(nc) as tc:
        with (
            tc.tile_pool(name="qpool", bufs=5) as qpool,
            tc.tile_pool(name="smpool", bufs=2) as smpool,
            tc.tile_pool(name="pspool", bufs=2, space="PSUM") as pspool,
            tc.tile_pool(name="tk", bufs=1) as tk,
        ):
            ones = tk.tile([P, 1], f32)
            nc.vector.memset(ones[:], 1.0)

            def topk_and_gather(b, sums):
                """Two-round top-16 on DVE; gathers issue per round via
                dynamic-offset HWDGE DMAs straight DRAM->DRAM."""
                gidx_a = tk.tile([1, 8], u32, name=f"gidxa{b}", tag="gidxa", bufs=2)
                gidx_b = tk.tile([1, 8], u32, name=f"gidxb{b}", tag="gidxb", bufs=2)
                m8a = tk.tile([1, 8], f32, name=f"m8a{b}", tag="m8a", bufs=2)
                m8b = tk.tile([1, 8], f32, name=f"m8b{b}", tag="m8b", bufs=2)
                nc.vector.max(m8a[:], sums[:])
                nc.vector.max_index(gidx_a[:], m8a[:], sums[:])

                def gather(k, gidx, eng, etype):
                    regs = nc.alloc_registers(name=f"ri{b}_{k}", engines=(etype,))
                    reg = list(regs)[0]
                    eng.reg_load(reg, gidx[0:1, k % 8 : k % 8 + 1])
                    val = eng.snap(reg, donate=True, min_val=0, max_val=S - 1)
                    eng.dma_start(
                        out_d[b * K + k : b * K + k + 1, :],
                        xt_d[b][bass.ds(val, 1), :],
                    )

                # on scalar (HWDGE) so the sync w-load queue never stalls
                # behind a top-k-dependent reg_load; the final batch has no
                # w stream left, so its gathers split onto the idle sync
                # queue too to halve the tail.
                def dispatch(k, gidx):
                    if b == B_LOC - 1 and k % 2 == 1:
                        gather(k, gidx, nc.sync, mybir.EngineType.SP)
                    else:
                        gather(k, gidx, nc.scalar, mybir.EngineType.Activation)

                for k in range(8):
                    dispatch(k, gidx_a)

                nc.vector.match_replace(sums[:], m8a[:], sums[:], NEG)
                nc.vector.max(m8b[:], sums[:])
                nc.vector.max_index(gidx_b[:], m8b[:], sums[:])
                for k in range(8, 16):
                    dispatch(k, gidx_b)

            prev = None  # (b, sums) whose top-k is deferred one batch
            for b in range(B_LOC):
                # --- stream w[b]: 16 x 1 MiB loads on the sync queue ---
                w0 = [
                    qpool.tile([P, S], f32, name=f"w0_{b}_{fr}", tag="w0")
                    for fr in range(FR)
                ]
                w1 = [
                    qpool.tile([P, S], f32, name=f"w1_{b}_{fr}", tag="w1")
                    for fr in range(FR)
                ]
                w2 = [
                    qpool.tile([P, S], f32, name=f"w2_{b}_{fr}", tag="w2")
                    for fr in range(FR)
                ]
                sts = [
                    qpool.tile([P, S], f32, name=f"st{b}_{m}", tag="st")
                    for m in range(4)
                ]
                for fr in range(FR):
                    nc.sync.dma_start(w0[fr][:], w_q[4 * b + 0, fr])
                for fr in range(FR):
                    nc.sync.dma_start(w1[fr][:], w_q[4 * b + 1, fr])
                for fr in range(FR):
                    nc.sync.dma_start(w2[fr][:], w_q[4 * b + 2, fr])
                for m in range(4):
                    nc.sync.dma_start(sts[m][:], w_small[b * 16 + 12 + m])

                # --- DVE chunk-adds, pipelined behind the quarter DMAs ---
                # (element-wise identical to acc = w0+w1; acc += w2;
                # offloading adds to gpsimd was tried and loses: its ops are
                # 2x slower and concurrent DVE+GpSimd elementwise contend
                # for SBUF ports, slowing both)
                for fr in range(FR):
                    nc.vector.tensor_add(w0[fr][:], w0[fr][:], w1[fr][:])
                for fr in range(FR):
                    nc.vector.tensor_add(w0[fr][:], w0[fr][:], w2[fr][:])

                # previous batch's top-k goes on the vector queue only now,
                # behind this batch's adds: its wait for the previous MM
                # chain must not head-of-line-block the adds that gate this
                # batch's matmuls (and hence the buffer frees the DMA
                # stream needs).
                if prev is not None:
                    topk_and_gather(*prev)

                ps = [
                    pspool.tile([1, MM_N], f32, name=f"ps{b}_{q}", tag=f"ps{q}")
                    for q in range(NQ)
                ]
                # single accumulation group per psum slice; WAW deps on the
                # psum AP keep the start=True matmul first
                for c in range(FR * NQ):
                    fr, q = c // NQ, c % NQ
                    nc.tensor.matmul(
                        ps[q][:],
                        ones[:],
                        w0[fr][:, q * MM_N : (q + 1) * MM_N],
                        start=(c < NQ),
                        stop=False,
                    )
                for m, st in enumerate(sts):
                    for q in range(NQ):
                        nc.tensor.matmul(
                            ps[q][:],
                            ones[:],
                            st[:, q * MM_N : (q + 1) * MM_N],
                            start=False,
                            stop=(m == 3),
                        )
    
                # PSUM -> column sums in SBUF
                sums = smpool.tile([1, S], f32, name=f"sums{b}", tag="sums")
                for q in range(NQ):
                    nc.scalar.activation(
                        sums[:, q * MM_N : (q + 1) * MM_N],
                        ps[q][:],
                        mybir.ActivationFunctionType.Copy,
                    )

                prev = (b, sums)

            # last batch's top-k + gathers are the kernel tail
            topk_and_gather(*prev)

    nc.compile()
    return nc


def _get_nc():
    global _cached_nc
    if _cached_nc is None:
        _cached_nc = _build_nc()
    return _cached_nc


def kernel(x: np.ndarray, w: np.ndarray) -> np.ndarray:
    from concourse import bass_utils

    x = np.asarray(x, dtype=np.float32)
    w = np.asarray(w, dtype=np.float32)
    x_t = np.ascontiguousarray(x.transpose(0, 2, 1))  # [B, S, F]

    nc = _get_nc()
    in_maps = [
        {
            "w": np.ascontiguousarray(w[c * B_LOC : (c + 1) * B_LOC]),
            "xt": x_t[c * B_LOC : (c + 1) * B_LOC],
        }
        for c in range(N_CORES)
    ]
    res = bass_utils.run_bass_kernel_spmd(
        nc, in_maps, list(range(N_CORES)), trace=TRACE
    )
    global _last_results
    _last_results = res
    out = np.concatenate([res.results[c]["out"] for c in range(N_CORES)], axis=0)
    # [B*K, F] -> [B, K, F] -> [B, F, K]
    return np.ascontiguousarray(out.reshape(B, K, F).transpose(0, 2, 1))


# revision 23
# speedup vs baseline: 1.1425x; 1.1088x over previous
"""AttnTopKPool Trainium2 kernel.

reference:
    w_mean = mean(w, axis=1)          # [B, S, S] -> [B, S]
    idx    = top_k(w_mean, 16)        # [B, 16]
    out    = x[b, :, idx[b]]          # [B, F, 16]

Strategy (8 NeuronCores, batch-parallel, 4 batches each):
  - host: transpose x to x_t[b, s, f] so the device gather is a contiguous
    row gather; slice w and x_t per core.
  - device per batch (w[b] is 16 MiB, streamed once; memory-bound):
      * 16 uniform 1 MiB loads [128, 2048]: three "big slots" worth of
        quarter tiles (wt0q/wt1q/wt2q, rows 512t+4p+fr) plus four
        partition-major small tiles (rows 1536+128m+p).
      * DVE chunk-adds pipelined with the stream: wt0q[fr] += wt1q[fr]
        as wt1 quarters land, then += wt2q[fr] as wt2 quarters land.
        Element-wise order is bit-identical to the previous whole-tile
        adds (this matters: several batches have near-tied column sums
        whose ordering under fp32 rounding must reproduce the
        reference's top_k exactly).
      * column sums via TensorE fp32 ones-matmul into 4 PSUM banks,
        accumulation order identical to the reference-passing schedule:
        16 slices over the pre-added quarters, then 16 over smalls.
      * top-16 via DVE max8 / max_index / match_replace (two rounds);
        gathers for ranks 0-7 issue as soon as round 1 lands.
      * gather: per index, reg_load into an SP register and issue a
        dynamic-offset HWDGE DMA copying that 4 KiB row of x_t[b]
        straight DRAM->DRAM into the output row.
  - out per core: [64, 1024] = (b_loc*16 + k, f); host reassembles to
    [B, F, K].
"""

import numpy as np

B, F, S, K = 32, 1024, 2048, 16
N_CORES = 8
B_LOC = B // N_CORES  # 4
P = 128
MM_N = 512                 # fp32 moving-operand max / one PSUM bank
NQ = S // MM_N             # 4 psum column slices
FR = 4                     # w rows per partition in a big slot
NEG = -3.0e38              # below any column sum

_cached_nc = None

# test-only knobs (harness leaves these at defaults)
TRACE = False
_last_results = None


def _build_nc():
    from concourse import bacc, bass, mybir, tile

    f32 = mybir.dt.float32
    u32 = mybir.dt.uint32

    nc = bacc.Bacc("TRN2", target_bir_lowering=False, debug=False)

    w_d = nc.dram_tensor("w", [B_LOC, S, S], f32, kind="ExternalInput")
    xt_d = nc.dram_tensor("xt", [B_LOC, S, F], f32, kind="ExternalInput")
    out_d = nc.dram_tensor("out", [B_LOC * K, F], f32, kind="ExternalOutput")

    w_rows = w_d[:].rearrange("b r s -> (b r) s")
    # quarter view: [32, 4, 128, 2048]; [x, fr] partition p holds row 512x+4p+fr
    w_q = w_rows.rearrange("(x p fr) s -> x fr p s", p=P, fr=FR)
    # small view: [64, 128, 2048]; partition p of slot m holds row (128m + p)
    w_small = w_rows.rearrange("(m p) s -> m p s", p=P)

    with tile.TileContext(nc) as tc:
        with (
            tc.tile_pool(name="qpool", bufs=5) as qpool,
            tc.tile_pool(name="smpool", bufs=2) as smpool,
            tc.tile_pool(name="pspool", bufs=2, space="PSUM") as pspool,
            tc.tile_pool(name="tk", bufs=1) as tk,
        ):
            ones = tk.tile([P, 1], f32)
            nc.vector.memset(ones[:], 1.0)

            def topk_and_gather(b, sums):
                """Two-round top-16 on DVE; gathers issue per round via
                dynamic-offset HWDGE DMAs straight DRAM->DRAM."""
                gidx_a = tk.tile([1, 8], u32, name=f"gidxa{b}", tag="gidxa", bufs=2)
                gidx_b = tk.tile([1, 8], u32, name=f"gidxb{b}", tag="gidxb", bufs=2)
                m8a = tk.tile([1, 8], f32, name=f"m8a{b}", tag="m8a", bufs=2)
                m8b = tk.tile([1, 8], f32, name=f"m8b{b}", tag="m8b", bufs=2)
                nc.vector.max(m8a[:], sums[:])
                nc.vector.max_index(gidx_a[:], m8a[:], sums[:])

                def gather(k, gidx, eng, etype):
                    regs = nc.alloc_registers(name=f"ri{b}_{k}", engines=(etype,))
                    reg = list(regs)[0]
                    eng.reg_load(reg, gidx[0:1, k % 8 : k % 8 + 1])
                    val = eng.snap(reg, donate=True, min_val=0, max_val=S - 1)
                    eng.dma_start(
                        out_d[b * K + k : b * K + k + 1, :],
                        xt_d[b][bass.ds(val, 1), :],
                    )

                # on the otherwise-idle gpsimd queue so a top-k-dependent
                # reg_load never head-of-line-blocks either w-load queue;
                # the final batch has no w stream left, so its gathers
                # split across all three queues to shrink the tail.
                def dispatch(k, gidx):
                    if b == B_LOC - 1 and k % 3 == 1:
                        gather(k, gidx, nc.sync, mybir.EngineType.SP)
                    elif b == B_LOC - 1 and k % 3 == 0:
                        gather(k, gidx, nc.scalar, mybir.EngineType.Activation)
                    else:
                        gather(k, gidx, nc.gpsimd, mybir.EngineType.Pool)

                for k in range(8):
                    dispatch(k, gidx_a)

                nc.vector.match_replace(sums[:], m8a[:], sums[:], NEG)
                nc.vector.max(m8b[:], sums[:])
                nc.vector.max_index(gidx_b[:], m8b[:], sums[:])
                for k in range(8, 16):
                    dispatch(k, gidx_b)

            prev = None  # (b, sums) whose top-k is deferred one batch
            for b in range(B_LOC):
                # --- stream w[b]: 16 x 1 MiB loads on the sync queue ---
                w0 = [
                    qpool.tile([P, S], f32, name=f"w0_{b}_{fr}", tag="w0")
                    for fr in range(FR)
                ]
                w1 = [
                    qpool.tile([P, S], f32, name=f"w1_{b}_{fr}", tag="w1")
                    for fr in range(FR)
                ]
                w2 = [
                    qpool.tile([P, S], f32, name=f"w2_{b}_{fr}", tag="w2")
                    for fr in range(FR)
                ]
                sts = [
                    qpool.tile([P, S], f32, name=f"st{b}_{m}", tag="st")
                    for m in range(4)
                ]
                # split the stream over both HWDGE queues: a buffer-free
                # wait on one queue then only stalls half the stream
                # (issue-queue head-of-line), and w2 lands a half-window
                # earlier, giving the add2->matmul chain more slack.
                for fr in range(FR):
                    nc.sync.dma_start(w0[fr][:], w_q[4 * b + 0, fr])
                for fr in range(FR):
                    nc.scalar.dma_start(w1[fr][:], w_q[4 * b + 1, fr])
                for fr in range(FR):
                    nc.sync.dma_start(w2[fr][:], w_q[4 * b + 2, fr])
                for m in range(4):
                    nc.scalar.dma_start(sts[m][:], w_small[b * 16 + 12 + m])

                # --- DVE chunk-adds, pipelined behind the quarter DMAs ---
                # (element-wise identical to acc = w0+w1; acc += w2;
                # offloading adds to gpsimd was tried and loses: its ops are
                # 2x slower and concurrent DVE+GpSimd elementwise contend
                # for SBUF ports, slowing both)
                for fr in range(FR):
                    nc.vector.tensor_add(w0[fr][:], w0[fr][:], w1[fr][:])
                for fr in range(FR):
                    nc.vector.tensor_add(w0[fr][:], w0[fr][:], w2[fr][:])

                # previous batch's top-k goes on the vector queue only now,
                # behind this batch's adds: its wait for the previous MM
                # chain must not head-of-line-block the adds that gate this
                # batch's matmuls (and hence the buffer frees the DMA
                # stream needs).
                if prev is not None:
                    topk_and_gather(*prev)

                ps = [
                    pspool.tile([1, MM_N], f32, name=f"ps{b}_{q}", tag=f"ps{q}")
                    for q in range(NQ)
                ]
                # single accumulation group per psum slice; WAW deps on the
                # psum AP keep the start=True matmul first
                for c in range(FR * NQ):
                    fr, q = c // NQ, c % NQ
                    nc.tensor.matmul(
                        ps[q][:],
                        ones[:],
                        w0[fr][:, q * MM_N : (q + 1) * MM_N],
                        start=(c < NQ),
                        stop=False,
                    )
                for m, st in enumerate(sts):
                    for q in range(NQ):
                        nc.tensor.matmul(
                            ps[q][:],
                            ones[:],
                            st[:, q * MM_N : (q + 1) * MM_N],
                            start=False,
                            stop=(m == 3),
                        )
    
                # PSUM -> column sums in SBUF
                sums = smpool.tile([1, S], f32, name=f"sums{b}", tag="sums")
                for q in range(NQ):
                    nc.scalar.activation(
                        sums[:, q * MM_N : (q + 1) * MM_N],
                        ps[q][:],
                        mybir.ActivationFunctionType.Copy,
                    )

                prev = (b, sums)

            # last batch's top-k + gathers are the kernel tail
            topk_and_gather(*prev)

    nc.compile()
    return nc


def _get_nc():
    global _cached_nc
    if _cached_nc is None:
        _cached_nc = _build_nc()
    return _cached_nc


def kernel(x: np.ndarray, w: np.ndarray) -> np.ndarray:
    from concourse import bass_utils

    x = np.asarray(x, dtype=np.float32)
    w = np.asarray(w, dtype=np.float32)
    x_t = np.ascontiguousarray(x.transpose(0, 2, 1))  # [B, S, F]

    nc = _get_nc()
    in_maps = [
        {
            "w": np.ascontiguousarray(w[c * B_LOC : (c + 1) * B_LOC]),
            "xt": x_t[c * B_LOC : (c + 1) * B_LOC],
        }
        for c in range(N_CORES)
    ]
    res = bass_utils.run_bass_kernel_spmd(
        nc, in_maps, list(range(N_CORES)), trace=TRACE
    )
    global _last_results
    _last_results = res
    out = np.concatenate([res.results[c]["out"] for c in range(N_CORES)], axis=0)
    # [B*K, F] -> [B, K, F] -> [B, F, K]
    return np.ascontiguousarray(out.reshape(B, K, F).transpose(0, 2, 1))


# revision 25
# speedup vs baseline: 1.1987x; 1.0491x over previous
"""AttnTopKPool Trainium2 kernel.

reference:
    w_mean = mean(w, axis=1)          # [B, S, S] -> [B, S]
    idx    = top_k(w_mean, 16)        # [B, 16]
    out    = x[b, :, idx[b]]          # [B, F, 16]

Strategy (8 NeuronCores, batch-parallel, 4 batches each):
  - host: transpose x to x_t[b, s, f] so the device gather is a contiguous
    row gather; slice w and x_t per core.
  - device per batch (w[b] is 16 MiB, streamed once; memory-bound):
      * 16 uniform 1 MiB loads [128, 2048]: three "big slots" worth of
        quarter tiles (wt0q/wt1q/wt2q, rows 512t+4p+fr) plus four
        partition-major small tiles (rows 1536+128m+p).
      * DVE chunk-adds pipelined with the stream: wt0q[fr] += wt1q[fr]
        as wt1 quarters land, then += wt2q[fr] as wt2 quarters land.
        Element-wise order is bit-identical to the previous whole-tile
        adds (this matters: several batches have near-tied column sums
        whose ordering under fp32 rounding must reproduce the
        reference's top_k exactly).
      * column sums via TensorE fp32 ones-matmul into 4 PSUM banks,
        accumulation order identical to the reference-passing schedule:
        16 slices over the pre-added quarters, then 16 over smalls.
      * top-16 via DVE max8 / max_index / match_replace (two rounds);
        gathers for ranks 0-7 issue as soon as round 1 lands.
      * gather: per index, reg_load into an SP register and issue a
        dynamic-offset HWDGE DMA copying that 4 KiB row of x_t[b]
        straight DRAM->DRAM into the output row.
  - out per core: [64, 1024] = (b_loc*16 + k, f); host reassembles to
    [B, F, K].
"""

import numpy as np

B, F, S, K = 32, 1024, 2048, 16
N_CORES = 8
B_LOC = B // N_CORES  # 4
P = 128
MM_N = 512                 # fp32 moving-operand max / one PSUM bank
NQ = S // MM_N             # 4 psum column slices
FR = 4                     # w rows per partition in a big slot
NEG = -3.0e38              # below any column sum

_cached_nc = None

# test-only knobs (harness leaves these at defaults)
TRACE = False
_last_results = None


def _build_nc():
    from concourse import bacc, bass, mybir, tile

    f32 = mybir.dt.float32
    u32 = mybir.dt.uint32

    nc = bacc.Bacc("TRN2", target_bir_lowering=False, debug=False)

    w_d = nc.dram_tensor("w", [B_LOC, S, S], f32, kind="ExternalInput")
    xt_d = nc.dram_tensor("xt", [B_LOC, S, F], f32, kind="ExternalInput")
    out_d = nc.dram_tensor("out", [B_LOC * K, F], f32, kind="ExternalOutput")

    w_rows = w_d[:].rearrange("b r s -> (b r) s")
    # quarter view: [32, 4, 128, 2048]; [x, fr] partition p holds row 512x+4p+fr
    w_q = w_rows.rearrange("(x p fr) s -> x fr p s", p=P, fr=FR)
    # small view: [64, 128, 2048]; partition p of slot m holds row (128m + p)
    w_small = w_rows.rearrange("(m p) s -> m p s", p=P)

    with tile.TileContext(nc) as tc:
        with (
            tc.tile_pool(name="qpool", bufs=5) as qpool,
            tc.tile_pool(name="smpool", bufs=2) as smpool,
            tc.tile_pool(name="pspool", bufs=2, space="PSUM") as pspool,
            tc.tile_pool(name="tk", bufs=1) as tk,
        ):
            ones = tk.tile([P, 1], f32)
            nc.vector.memset(ones[:], 1.0)

            def topk_and_gather(b, sums):
                """Two-round top-16 on DVE; gathers issue per round via
                dynamic-offset HWDGE DMAs straight DRAM->DRAM."""
                gidx_a = tk.tile([1, 8], u32, name=f"gidxa{b}", tag="gidxa", bufs=2)
                gidx_b = tk.tile([1, 8], u32, name=f"gidxb{b}", tag="gidxb", bufs=2)
                m8a = tk.tile([1, 8], f32, name=f"m8a{b}", tag="m8a", bufs=2)
                m8b = tk.tile([1, 8], f32, name=f"m8b{b}", tag="m8b", bufs=2)
                nc.vector.max(m8a[:], sums[:])
                nc.vector.max_index(gidx_a[:], m8a[:], sums[:])

                def gather(k, gidx, eng, etype):
                    regs = nc.alloc_registers(name=f"ri{b}_{k}", engines=(etype,))
                    reg = list(regs)[0]
                    eng.reg_load(reg, gidx[0:1, k % 8 : k % 8 + 1])
                    val = eng.snap(reg, donate=True, min_val=0, max_val=S - 1)
                    eng.dma_start(
                        out_d[b * K + k : b * K + k + 1, :],
                        xt_d[b][bass.ds(val, 1), :],
                    )

                # on the otherwise-idle gpsimd queue so a top-k-dependent
                # reg_load never head-of-line-blocks either w-load queue;
                # the final batch has no w stream left, so its gathers
                # split across all three queues to shrink the tail.
                def dispatch(k, gidx):
                    if b == B_LOC - 1 and k % 3 == 1:
                        gather(k, gidx, nc.sync, mybir.EngineType.SP)
                    elif b == B_LOC - 1 and k % 3 == 0:
                        gather(k, gidx, nc.scalar, mybir.EngineType.Activation)
                    else:
                        gather(k, gidx, nc.gpsimd, mybir.EngineType.Pool)

                for k in range(8):
                    dispatch(k, gidx_a)

                nc.vector.match_replace(sums[:], m8a[:], sums[:], NEG)
                nc.vector.max(m8b[:], sums[:])
                nc.vector.max_index(gidx_b[:], m8b[:], sums[:])
                for k in range(8, 16):
                    dispatch(k, gidx_b)

            prev = None  # (b, sums) whose top-k is deferred one batch
            for b in range(B_LOC):
                # --- stream w[b]: 16 x 1 MiB loads on the sync queue ---
                w0 = [
                    qpool.tile([P, S], f32, name=f"w0_{b}_{fr}", tag="w0")
                    for fr in range(FR)
                ]
                w1 = [
                    qpool.tile([P, S], f32, name=f"w1_{b}_{fr}", tag="w1")
                    for fr in range(FR)
                ]
                w2 = [
                    qpool.tile([P, S], f32, name=f"w2_{b}_{fr}", tag="w2")
                    for fr in range(FR)
                ]
                sts = [
                    qpool.tile([P, S], f32, name=f"st{b}_{m}", tag="st")
                    for m in range(4)
                ]
                # split the stream over both HWDGE queues: a buffer-free
                # wait on one queue then only stalls half the stream
                # (issue-queue head-of-line), and w2 lands a half-window
                # earlier, giving the add2->matmul chain more slack. For
                # the final batch, land all w2 quarters mid-stream and the
                # small tiles last, so the acc matmul chain completes
                # during the stream and only the small-tile matmuls remain
                # in the tail.
                if b < B_LOC - 1:
                    for fr in range(FR):
                        nc.sync.dma_start(w0[fr][:], w_q[4 * b + 0, fr])
                    for fr in range(FR):
                        nc.scalar.dma_start(w1[fr][:], w_q[4 * b + 1, fr])
                    for fr in range(FR):
                        nc.sync.dma_start(w2[fr][:], w_q[4 * b + 2, fr])
                    for m in range(4):
                        nc.scalar.dma_start(sts[m][:], w_small[b * 16 + 12 + m])
                else:
                    for fr in range(FR):
                        nc.sync.dma_start(w0[fr][:], w_q[4 * b + 0, fr])
                    for fr in range(FR):
                        nc.scalar.dma_start(w1[fr][:], w_q[4 * b + 1, fr])
                    nc.scalar.dma_start(w2[0][:], w_q[4 * b + 2, 0])
                    nc.scalar.dma_start(w2[1][:], w_q[4 * b + 2, 1])
                    nc.sync.dma_start(w2[2][:], w_q[4 * b + 2, 2])
                    nc.sync.dma_start(w2[3][:], w_q[4 * b + 2, 3])
                    nc.sync.dma_start(sts[0][:], w_small[b * 16 + 12 + 0])
                    nc.scalar.dma_start(sts[1][:], w_small[b * 16 + 12 + 1])
                    nc.sync.dma_start(sts[2][:], w_small[b * 16 + 12 + 2])
                    nc.scalar.dma_start(sts[3][:], w_small[b * 16 + 12 + 3])

                # --- DVE chunk-adds, pipelined behind the quarter DMAs ---
                # (element-wise identical to acc = w0+w1; acc += w2;
                # offloading adds to gpsimd was tried and loses: its ops are
                # 2x slower and concurrent DVE+GpSimd elementwise contend
                # for SBUF ports, slowing both)
                for fr in range(FR):
                    nc.vector.tensor_add(w0[fr][:], w0[fr][:], w1[fr][:])

                # previous batch's top-k sits between add1 (gated by the
                # early w1 landings) and add2 (gated by the late w2
                # landings) on the vector queue: it fills DVE's natural
                # mid-window gap instead of spilling past the stream, and
                # its wait for the previous MM chain (long done) cannot
                # head-of-line-block anything.
                if prev is not None:
                    topk_and_gather(*prev)

                for fr in range(FR):
                    nc.vector.tensor_add(w0[fr][:], w0[fr][:], w2[fr][:])

                ps = [
                    pspool.tile([1, MM_N], f32, name=f"ps{b}_{q}", tag=f"ps{q}")
                    for q in range(NQ)
                ]
                # single accumulation group per psum slice; WAW deps on the
                # psum AP keep the start=True matmul first
                for c in range(FR * NQ):
                    fr, q = c // NQ, c % NQ
                    nc.tensor.matmul(
                        ps[q][:],
                        ones[:],
                        w0[fr][:, q * MM_N : (q + 1) * MM_N],
                        start=(c < NQ),
                        stop=False,
                    )
                for m, st in enumerate(sts):
                    for q in range(NQ):
                        nc.tensor.matmul(
                            ps[q][:],
                            ones[:],
                            st[:, q * MM_N : (q + 1) * MM_N],
                            start=False,
                            stop=(m == 3),
                        )
    
                # PSUM -> column sums in SBUF
                sums = smpool.tile([1, S], f32, name=f"sums{b}", tag="sums")
                for q in range(NQ):
                    nc.scalar.activation(
                        sums[:, q * MM_N : (q + 1) * MM_N],
                        ps[q][:],
                        mybir.ActivationFunctionType.Copy,
                    )

                prev = (b, sums)

            # last batch's top-k + gathers are the kernel tail
            topk_and_gather(*prev)

    nc.compile()
    return nc


def _get_nc():
    global _cached_nc
    if _cached_nc is None:
        _cached_nc = _build_nc()
    return _cached_nc


def kernel(x: np.ndarray, w: np.ndarray) -> np.ndarray:
    from concourse import bass_utils

    x = np.asarray(x, dtype=np.float32)
    w = np.asarray(w, dtype=np.float32)
    x_t = np.ascontiguousarray(x.transpose(0, 2, 1))  # [B, S, F]

    nc = _get_nc()
    in_maps = [
        {
            "w": np.ascontiguousarray(w[c * B_LOC : (c + 1) * B_LOC]),
            "xt": x_t[c * B_LOC : (c + 1) * B_LOC],
        }
        for c in range(N_CORES)
    ]
    res = bass_utils.run_bass_kernel_spmd(
        nc, in_maps, list(range(N_CORES)), trace=TRACE
    )
    global _last_results
    _last_results = res
    out = np.concatenate([res.results[c]["out"] for c in range(N_CORES)], axis=0)
    # [B*K, F] -> [B, K, F] -> [B, F, K]
    return np.ascontiguousarray(out.reshape(B, K, F).transpose(0, 2, 1))


# revision 26
# speedup vs baseline: 1.2019x; 1.0027x over previous
"""AttnTopKPool Trainium2 kernel.

reference:
    w_mean = mean(w, axis=1)          # [B, S, S] -> [B, S]
    idx    = top_k(w_mean, 16)        # [B, 16]
    out    = x[b, :, idx[b]]          # [B, F, 16]

Strategy (8 NeuronCores, batch-parallel, 4 batches each):
  - host: transpose x to x_t[b, s, f] so the device gather is a contiguous
    row gather; slice w and x_t per core.
  - device per batch (w[b] is 16 MiB, streamed once; memory-bound):
      * 16 uniform 1 MiB loads [128, 2048]: three "big slots" worth of
        quarter tiles (wt0q/wt1q/wt2q, rows 512t+4p+fr) plus four
        partition-major small tiles (rows 1536+128m+p).
      * DVE chunk-adds pipelined with the stream: wt0q[fr] += wt1q[fr]
        as wt1 quarters land, then += wt2q[fr] as wt2 quarters land.
        Element-wise order is bit-identical to the previous whole-tile
        adds (this matters: several batches have near-tied column sums
        whose ordering under fp32 rounding must reproduce the
        reference's top_k exactly).
      * column sums via TensorE fp32 ones-matmul into 4 PSUM banks,
        accumulation order identical to the reference-passing schedule:
        16 slices over the pre-added quarters, then 16 over smalls.
      * top-16 via DVE max8 / max_index / match_replace (two rounds);
        gathers for ranks 0-7 issue as soon as round 1 lands.
      * gather: per index, reg_load into an SP register and issue a
        dynamic-offset HWDGE DMA copying that 4 KiB row of x_t[b]
        straight DRAM->DRAM into the output row.
  - out per core: [64, 1024] = (b_loc*16 + k, f); host reassembles to
    [B, F, K].
"""

import numpy as np

B, F, S, K = 32, 1024, 2048, 16
N_CORES = 8
B_LOC = B // N_CORES  # 4
P = 128
MM_N = 512                 # fp32 moving-operand max / one PSUM bank
NQ = S // MM_N             # 4 psum column slices
FR = 4                     # w rows per partition in a big slot
NEG = -3.0e38              # below any column sum

_cached_nc = None

# test-only knobs (harness leaves these at defaults)
TRACE = False
_last_results = None


def _build_nc():
    from concourse import bacc, bass, mybir, tile

    f32 = mybir.dt.float32
    u32 = mybir.dt.uint32

    nc = bacc.Bacc("TRN2", target_bir_lowering=False, debug=False)

    w_d = nc.dram_tensor("w", [B_LOC, S, S], f32, kind="ExternalInput")
    xt_d = nc.dram_tensor("xt", [B_LOC, S, F], f32, kind="ExternalInput")
    out_d = nc.dram_tensor("out", [B_LOC * K, F], f32, kind="ExternalOutput")

    w_rows = w_d[:].rearrange("b r s -> (b r) s")
    # quarter view: [32, 4, 128, 2048]; [x, fr] partition p holds row 512x+4p+fr
    w_q = w_rows.rearrange("(x p fr) s -> x fr p s", p=P, fr=FR)
    # small view: [64, 128, 2048]; partition p of slot m holds row (128m + p)
    w_small = w_rows.rearrange("(m p) s -> m p s", p=P)

    with tile.TileContext(nc) as tc:
        with (
            tc.tile_pool(name="qpool", bufs=5) as qpool,
            tc.tile_pool(name="smpool", bufs=2) as smpool,
            tc.tile_pool(name="pspool", bufs=2, space="PSUM") as pspool,
            tc.tile_pool(name="tk", bufs=1) as tk,
        ):
            ones = tk.tile([P, 1], f32)
            nc.vector.memset(ones[:], 1.0)

            def topk_and_gather(b, sums):
                """Two-round top-16 on DVE; gathers issue per round via
                dynamic-offset HWDGE DMAs straight DRAM->DRAM."""
                gidx_a = tk.tile([1, 8], u32, name=f"gidxa{b}", tag="gidxa", bufs=2)
                gidx_b = tk.tile([1, 8], u32, name=f"gidxb{b}", tag="gidxb", bufs=2)
                m8a = tk.tile([1, 8], f32, name=f"m8a{b}", tag="m8a", bufs=2)
                m8b = tk.tile([1, 8], f32, name=f"m8b{b}", tag="m8b", bufs=2)
                nc.vector.max(m8a[:], sums[:])
                nc.vector.max_index(gidx_a[:], m8a[:], sums[:])

                def gather(k, gidx, eng, etype):
                    regs = nc.alloc_registers(name=f"ri{b}_{k}", engines=(etype,))
                    reg = list(regs)[0]
                    eng.reg_load(reg, gidx[0:1, k % 8 : k % 8 + 1])
                    val = eng.snap(reg, donate=True, min_val=0, max_val=S - 1)
                    eng.dma_start(
                        out_d[b * K + k : b * K + k + 1, :],
                        xt_d[b][bass.ds(val, 1), :],
                    )

                # on the otherwise-idle gpsimd queue so a top-k-dependent
                # reg_load never head-of-line-blocks either w-load queue;
                # the final batch has no w stream left, so its gathers
                # split across all three queues to shrink the tail.
                def dispatch(k, gidx):
                    if b == B_LOC - 1 and k % 3 == 1:
                        gather(k, gidx, nc.sync, mybir.EngineType.SP)
                    elif b == B_LOC - 1 and k % 3 == 0:
                        gather(k, gidx, nc.scalar, mybir.EngineType.Activation)
                    else:
                        gather(k, gidx, nc.gpsimd, mybir.EngineType.Pool)

                for k in range(8):
                    dispatch(k, gidx_a)

                nc.vector.match_replace(sums[:], m8a[:], sums[:], NEG)
                nc.vector.max(m8b[:], sums[:])
                nc.vector.max_index(gidx_b[:], m8b[:], sums[:])
                for k in range(8, 16):
                    dispatch(k, gidx_b)

            prev = None  # (b, sums) whose top-k is deferred one batch
            for b in range(B_LOC):
                # --- stream w[b]: 16 x 1 MiB loads on the sync queue ---
                w0 = [
                    qpool.tile([P, S], f32, name=f"w0_{b}_{fr}", tag="w0")
                    for fr in range(FR)
                ]
                w1 = [
                    qpool.tile([P, S], f32, name=f"w1_{b}_{fr}", tag="w1")
                    for fr in range(FR)
                ]
                w2 = [
                    qpool.tile([P, S], f32, name=f"w2_{b}_{fr}", tag="w2")
                    for fr in range(FR)
                ]
                sts = [
                    qpool.tile([P, S], f32, name=f"st{b}_{m}", tag="st")
                    for m in range(4)
                ]
                # split the stream over both HWDGE queues: a buffer-free
                # wait on one queue then only stalls half the stream
                # (issue-queue head-of-line), and w2 lands a half-window
                # earlier, giving the add2->matmul chain more slack. For
                # the final batch, land all w2 quarters mid-stream and the
                # small tiles last, so the acc matmul chain completes
                # during the stream and only the small-tile matmuls remain
                # in the tail.
                if b < B_LOC - 1:
                    for fr in range(FR):
                        nc.sync.dma_start(w0[fr][:], w_q[4 * b + 0, fr])
                    for fr in range(FR):
                        nc.scalar.dma_start(w1[fr][:], w_q[4 * b + 1, fr])
                    for fr in range(FR):
                        nc.sync.dma_start(w2[fr][:], w_q[4 * b + 2, fr])
                    for m in range(4):
                        nc.scalar.dma_start(sts[m][:], w_small[b * 16 + 12 + m])
                else:
                    for fr in range(FR):
                        nc.sync.dma_start(w0[fr][:], w_q[4 * b + 0, fr])
                    for fr in range(FR):
                        nc.scalar.dma_start(w1[fr][:], w_q[4 * b + 1, fr])
                    nc.scalar.dma_start(w2[0][:], w_q[4 * b + 2, 0])
                    nc.scalar.dma_start(w2[1][:], w_q[4 * b + 2, 1])
                    nc.sync.dma_start(w2[2][:], w_q[4 * b + 2, 2])
                    nc.sync.dma_start(w2[3][:], w_q[4 * b + 2, 3])
                    nc.sync.dma_start(sts[0][:], w_small[b * 16 + 12 + 0])
                    nc.scalar.dma_start(sts[1][:], w_small[b * 16 + 12 + 1])
                    nc.sync.dma_start(sts[2][:], w_small[b * 16 + 12 + 2])
                    nc.scalar.dma_start(sts[3][:], w_small[b * 16 + 12 + 3])

                # --- DVE chunk-adds, pipelined behind the quarter DMAs ---
                # (element-wise identical to acc = w0+w1; acc += w2;
                # offloading adds to gpsimd was tried and loses: its ops are
                # 2x slower and concurrent DVE+GpSimd elementwise contend
                # for SBUF ports, slowing both)
                for fr in range(FR):
                    nc.vector.tensor_add(w0[fr][:], w0[fr][:], w1[fr][:])

                # previous batch's top-k sits between add1 (gated by the
                # early w1 landings) and add2 (gated by the late w2
                # landings) on the vector queue: it fills DVE's natural
                # mid-window gap instead of spilling past the stream, and
                # its wait for the previous MM chain (long done) cannot
                # head-of-line-block anything. For the final batch the
                # priorities flip: add2 gates the tail's matmul chain and
                # post-stream DVE time is free, so the deferred top-k goes
                # after the adds instead.
                if prev is not None and b < B_LOC - 1:
                    topk_and_gather(*prev)

                for fr in range(FR):
                    nc.vector.tensor_add(w0[fr][:], w0[fr][:], w2[fr][:])

                if prev is not None and b == B_LOC - 1:
                    topk_and_gather(*prev)

                ps = [
                    pspool.tile([1, MM_N], f32, name=f"ps{b}_{q}", tag=f"ps{q}")
                    for q in range(NQ)
                ]
                # single accumulation group per psum slice; WAW deps on the
                # psum AP keep the start=True matmul first
                for c in range(FR * NQ):
                    fr, q = c // NQ, c % NQ
                    nc.tensor.matmul(
                        ps[q][:],
                        ones[:],
                        w0[fr][:, q * MM_N : (q + 1) * MM_N],
                        start=(c < NQ),
                        stop=False,
                    )
                for m, st in enumerate(sts):
                    for q in range(NQ):
                        nc.tensor.matmul(
                            ps[q][:],
                            ones[:],
                            st[:, q * MM_N : (q + 1) * MM_N],
                            start=False,
                            stop=(m == 3),
                        )
    
                # PSUM -> column sums in SBUF
                sums = smpool.tile([1, S], f32, name=f"sums{b}", tag="sums")
                for q in range(NQ):
                    nc.scalar.activation(
                        sums[:, q * MM_N : (q + 1) * MM_N],
                        ps[q][:],
                        mybir.ActivationFunctionType.Copy,
                    )

                prev = (b, sums)

            # last batch's top-k + gathers are the kernel tail
            topk_and_gather(*prev)

    nc.compile()
    return nc


def _get_nc():
    global _cached_nc
    if _cached_nc is None:
        _cached_nc = _build_nc()
    return _cached_nc


def kernel(x: np.ndarray, w: np.ndarray) -> np.ndarray:
    from concourse import bass_utils

    x = np.asarray(x, dtype=np.float32)
    w = np.asarray(w, dtype=np.float32)
    x_t = np.ascontiguousarray(x.transpose(0, 2, 1))  # [B, S, F]

    nc = _get_nc()
    in_maps = [
        {
            "w": np.ascontiguousarray(w[c * B_LOC : (c + 1) * B_LOC]),
            "xt": x_t[c * B_LOC : (c + 1) * B_LOC],
        }
        for c in range(N_CORES)
    ]
    res = bass_utils.run_bass_kernel_spmd(
        nc, in_maps, list(range(N_CORES)), trace=TRACE
    )
    global _last_results
    _last_results = res
    out = np.concatenate([res.results[c]["out"] for c in range(N_CORES)], axis=0)
    # [B*K, F] -> [B, K, F] -> [B, F, K]
    return np.ascontiguousarray(out.reshape(B, K, F).transpose(0, 2, 1))


# revision 28
# speedup vs baseline: 1.2051x; 1.0027x over previous
"""AttnTopKPool Trainium2 kernel.

reference:
    w_mean = mean(w, axis=1)          # [B, S, S] -> [B, S]
    idx    = top_k(w_mean, 16)        # [B, 16]
    out    = x[b, :, idx[b]]          # [B, F, 16]

Strategy (8 NeuronCores, batch-parallel, 4 batches each):
  - host: transpose x to x_t[b, s, f] so the device gather is a contiguous
    row gather; slice w and x_t per core.
  - device per batch (w[b] is 16 MiB, streamed once; memory-bound):
      * 16 uniform 1 MiB loads [128, 2048]: three "big slots" worth of
        quarter tiles (wt0q/wt1q/wt2q, rows 512t+4p+fr) plus four
        partition-major small tiles (rows 1536+128m+p).
      * DVE chunk-adds pipelined with the stream: wt0q[fr] += wt1q[fr]
        as wt1 quarters land, then += wt2q[fr] as wt2 quarters land.
        Element-wise order is bit-identical to the previous whole-tile
        adds (this matters: several batches have near-tied column sums
        whose ordering under fp32 rounding must reproduce the
        reference's top_k exactly).
      * column sums via TensorE fp32 ones-matmul into 4 PSUM banks,
        accumulation order identical to the reference-passing schedule:
        16 slices over the pre-added quarters, then 16 over smalls.
      * top-16 via DVE max8 / max_index / match_replace (two rounds);
        gathers for ranks 0-7 issue as soon as round 1 lands.
      * gather: per index, reg_load into an SP register and issue a
        dynamic-offset HWDGE DMA copying that 4 KiB row of x_t[b]
        straight DRAM->DRAM into the output row.
  - out per core: [64, 1024] = (b_loc*16 + k, f); host reassembles to
    [B, F, K].
"""

import numpy as np

B, F, S, K = 32, 1024, 2048, 16
N_CORES = 8
B_LOC = B // N_CORES  # 4
P = 128
MM_N = 512                 # fp32 moving-operand max / one PSUM bank
NQ = S // MM_N             # 4 psum column slices
FR = 4                     # w rows per partition in a big slot
NEG = -3.0e38              # below any column sum

_cached_nc = None

# test-only knobs (harness leaves these at defaults)
TRACE = False
_last_results = None


def _build_nc():
    from concourse import bacc, bass, mybir, tile

    f32 = mybir.dt.float32
    u32 = mybir.dt.uint32

    nc = bacc.Bacc("TRN2", target_bir_lowering=False, debug=False)

    w_d = nc.dram_tensor("w", [B_LOC, S, S], f32, kind="ExternalInput")
    xt_d = nc.dram_tensor("xt", [B_LOC, S, F], f32, kind="ExternalInput")
    out_d = nc.dram_tensor("out", [B_LOC * K, F], f32, kind="ExternalOutput")

    w_rows = w_d[:].rearrange("b r s -> (b r) s")
    # quarter view: [32, 4, 128, 2048]; [x, fr] partition p holds row 512x+4p+fr
    w_q = w_rows.rearrange("(x p fr) s -> x fr p s", p=P, fr=FR)
    # small view: [64, 128, 2048]; partition p of slot m holds row (128m + p)
    w_small = w_rows.rearrange("(m p) s -> m p s", p=P)

    with tile.TileContext(nc) as tc:
        with (
            tc.tile_pool(name="qpool", bufs=5) as qpool,
            tc.tile_pool(name="smpool", bufs=2) as smpool,
            tc.tile_pool(name="pspool", bufs=2, space="PSUM") as pspool,
            tc.tile_pool(name="tk", bufs=1) as tk,
        ):
            ones = tk.tile([P, 1], f32)
            nc.vector.memset(ones[:], 1.0)

            def topk_and_gather(b, sums):
                """Two-round top-16 on DVE; gathers issue per round via
                dynamic-offset HWDGE DMAs straight DRAM->DRAM."""
                gidx_a = tk.tile([1, 8], u32, name=f"gidxa{b}", tag="gidxa", bufs=2)
                gidx_b = tk.tile([1, 8], u32, name=f"gidxb{b}", tag="gidxb", bufs=2)
                m8a = tk.tile([1, 8], f32, name=f"m8a{b}", tag="m8a", bufs=2)
                m8b = tk.tile([1, 8], f32, name=f"m8b{b}", tag="m8b", bufs=2)
                nc.vector.max(m8a[:], sums[:])
                nc.vector.max_index(gidx_a[:], m8a[:], sums[:])

                def gather(k, gidx, eng, etype):
                    regs = nc.alloc_registers(name=f"ri{b}_{k}", engines=(etype,))
                    reg = list(regs)[0]
                    eng.reg_load(reg, gidx[0:1, k % 8 : k % 8 + 1])
                    val = eng.snap(reg, donate=True, min_val=0, max_val=S - 1)
                    eng.dma_start(
                        out_d[b * K + k : b * K + k + 1, :],
                        xt_d[b][bass.ds(val, 1), :],
                    )

                # on the otherwise-idle gpsimd queue so a top-k-dependent
                # reg_load never head-of-line-blocks either w-load queue;
                # the final batch has no w stream left, so its gathers
                # split across all three queues to shrink the tail.
                def dispatch(k, gidx):
                    if b == B_LOC - 1 and k % 3 == 1:
                        gather(k, gidx, nc.sync, mybir.EngineType.SP)
                    elif b == B_LOC - 1 and k % 3 == 0:
                        gather(k, gidx, nc.scalar, mybir.EngineType.Activation)
                    else:
                        gather(k, gidx, nc.gpsimd, mybir.EngineType.Pool)

                for k in range(8):
                    dispatch(k, gidx_a)

                nc.vector.match_replace(sums[:], m8a[:], sums[:], NEG)
                nc.vector.max(m8b[:], sums[:])
                nc.vector.max_index(gidx_b[:], m8b[:], sums[:])
                for k in range(8, 16):
                    dispatch(k, gidx_b)

            prev = None  # (b, sums) whose top-k is deferred one batch
            for b in range(B_LOC):
                # --- stream w[b]: 16 x 1 MiB loads on the sync queue ---
                w0 = [
                    qpool.tile([P, S], f32, name=f"w0_{b}_{fr}", tag="w0")
                    for fr in range(FR)
                ]
                w1 = [
                    qpool.tile([P, S], f32, name=f"w1_{b}_{fr}", tag="w1")
                    for fr in range(FR)
                ]
                w2 = [
                    qpool.tile([P, S], f32, name=f"w2_{b}_{fr}", tag="w2")
                    for fr in range(FR)
                ]
                sts = [
                    qpool.tile([P, S], f32, name=f"st{b}_{m}", tag="st")
                    for m in range(4)
                ]
                # split the stream over both HWDGE queues: a buffer-free
                # wait on one queue then only stalls half the stream
                # (issue-queue head-of-line), and w2 lands a half-window
                # earlier, giving the add2->matmul chain more slack. For
                # the final batch, land all w2 quarters mid-stream and the
                # small tiles last, so the acc matmul chain completes
                # during the stream and only the small-tile matmuls remain
                # in the tail.
                if b < B_LOC - 1:
                    for fr in range(FR):
                        nc.sync.dma_start(w0[fr][:], w_q[4 * b + 0, fr])
                    for fr in range(FR):
                        nc.scalar.dma_start(w1[fr][:], w_q[4 * b + 1, fr])
                    for fr in range(FR):
                        nc.sync.dma_start(w2[fr][:], w_q[4 * b + 2, fr])
                    for m in range(4):
                        nc.scalar.dma_start(sts[m][:], w_small[b * 16 + 12 + m])
                else:
                    for fr in range(FR):
                        nc.sync.dma_start(w0[fr][:], w_q[4 * b + 0, fr])
                    for fr in range(FR):
                        nc.scalar.dma_start(w1[fr][:], w_q[4 * b + 1, fr])
                    nc.scalar.dma_start(w2[0][:], w_q[4 * b + 2, 0])
                    nc.scalar.dma_start(w2[1][:], w_q[4 * b + 2, 1])
                    nc.sync.dma_start(w2[2][:], w_q[4 * b + 2, 2])
                    nc.sync.dma_start(w2[3][:], w_q[4 * b + 2, 3])
                    nc.sync.dma_start(sts[0][:], w_small[b * 16 + 12 + 0])
                    nc.scalar.dma_start(sts[1][:], w_small[b * 16 + 12 + 1])
                    nc.sync.dma_start(sts[2][:], w_small[b * 16 + 12 + 2])
                    nc.scalar.dma_start(sts[3][:], w_small[b * 16 + 12 + 3])

                # --- DVE chunk-adds, pipelined behind the quarter DMAs ---
                # (element-wise identical to acc = w0+w1; acc += w2;
                # offloading adds to gpsimd was tried and loses: its ops are
                # 2x slower and concurrent DVE+GpSimd elementwise contend
                # for SBUF ports, slowing both)
                for fr in range(FR):
                    nc.vector.tensor_add(w0[fr][:], w0[fr][:], w1[fr][:])

                # previous batch's top-k sits between add1 (gated by the
                # early w1 landings) and add2 (gated by the late w2
                # landings) on the vector queue: it fills DVE's natural
                # mid-window gap instead of spilling past the stream, and
                # its wait for the previous MM chain (long done) cannot
                # head-of-line-block anything. For the final batch the
                # priorities flip: add2 gates the tail's matmul chain and
                # post-stream DVE time is free, so the deferred top-k goes
                # after the adds instead.
                if prev is not None and b < B_LOC - 1:
                    topk_and_gather(*prev)

                for fr in range(FR):
                    nc.vector.tensor_add(w0[fr][:], w0[fr][:], w2[fr][:])

                if prev is not None and b == B_LOC - 1:
                    topk_and_gather(*prev)

                ps = [
                    pspool.tile([1, MM_N], f32, name=f"ps{b}_{q}", tag=f"ps{q}")
                    for q in range(NQ)
                ]
                # single accumulation group per psum slice; WAW deps on the
                # psum AP keep the start=True matmul first
                for c in range(FR * NQ):
                    fr, q = c // NQ, c % NQ
                    nc.tensor.matmul(
                        ps[q][:],
                        ones[:],
                        w0[fr][:, q * MM_N : (q + 1) * MM_N],
                        start=(c < NQ),
                        stop=False,
                    )
                for m, st in enumerate(sts):
                    for q in range(NQ):
                        nc.tensor.matmul(
                            ps[q][:],
                            ones[:],
                            st[:, q * MM_N : (q + 1) * MM_N],
                            start=False,
                            stop=(m == 3),
                        )
    
                # PSUM -> column sums in SBUF
                sums = smpool.tile([1, S], f32, name=f"sums{b}", tag="sums")
                for q in range(NQ):
                    nc.scalar.activation(
                        sums[:, q * MM_N : (q + 1) * MM_N],
                        ps[q][:],
                        mybir.ActivationFunctionType.Copy,
                    )

                prev = (b, sums)

            # last batch's top-k + gathers are the kernel tail
            topk_and_gather(*prev)

    nc.compile()
    return nc


def _get_nc():
    global _cached_nc
    if _cached_nc is None:
        _cached_nc = _build_nc()
    return _cached_nc


def kernel(x: np.ndarray, w: np.ndarray) -> np.ndarray:
    from concourse import bass_utils

    x = np.asarray(x, dtype=np.float32)
    w = np.asarray(w, dtype=np.float32)
    x_t = np.ascontiguousarray(x.transpose(0, 2, 1))  # [B, S, F]

    nc = _get_nc()
    in_maps = [
        {
            "w": np.ascontiguousarray(w[c * B_LOC : (c + 1) * B_LOC]),
            "xt": x_t[c * B_LOC : (c + 1) * B_LOC],
        }
        for c in range(N_CORES)
    ]
    res = bass_utils.run_bass_kernel_spmd(
        nc, in_maps, list(range(N_CORES)), trace=TRACE
    )
    global _last_results
    _last_results = res
    out = np.concatenate([res.results[c]["out"] for c in range(N_CORES)], axis=0)
    # [B*K, F] -> [B, K, F] -> [B, F, K]
    return np.ascontiguousarray(out.reshape(B, K, F).transpose(0, 2, 1))
